# revision 1
# baseline (speedup 1.0000x reference)
"""Trainium2 Bass kernel for nn_AccumulatingModule (histogram_binning).

Problem: out = score_matrix.at[qt, p, ol1, ol2].add(at1*at2) — a scatter-add of
BATCH*PAIR outer-product contributions into a [65, 90, 151, 151] fp32 histogram.

Strategy (8 NeuronCores, SPMD):
  * Memory roofline: stream score_matrix (533 MB) in + out once; everything
    else must hide under that.
  * Shard the (qt, pair) space: each qt's 90 pairs split into two 45-pair
    "half sections" (first-box-index i in {0..4} / {5..9}); 130 half
    sections + 6 dummies = 17 per core.
  * Box-permutation trick keeps the compiled kernel identical across cores
    (SPMD): every section computes the FIXED pattern pairs {(i,j): i in
    0..4, j != i}; the host permutes the 10 box columns per section and
    orders score rows to match the kernel's slot order.
  * W[b,k,:] = attention[b,k] * onehot(label[b,k]) built on GpSimd
    (tensor_scalar is_equal*mult vs an iota row), bf16 (one-hot exact, at
    rounded once -> ~2^-9 relative error on the sparse delta only).
  * delta[pair(i,j)] = W_j^T @ W_i on TensorE, PSUM-accumulated over two
    128-row chunks.  o1=151 -> 128-row main piece + 23-row tail; tails of
    4 groups share one PSUM bank at partition offsets 0/32/64/96 via
    matmul col-tiling.
  * The host pre-swizzles score into partition-major DRAM blocks
    (score_main [128, slots*151], score_tail [128, tailw] banded) so every
    score transfer is a 2D full-128-partition DMA — few dma_starts (HWDGE
    is a ~625ns/DMA serial resource) with large per-partition descriptors.
  * out = psum + score on VectorE, DMA'd back to the swizzled layout; host
    un-swizzles and scatters rows back.
"""

import numpy as np

NUM_QT, NUM_OT, PAIR = 65, 151, 90
BOX = 10
OT = NUM_OT
ROWLEN = OT * OT  # 22801
SECP = 45  # pairs per (half) section
NSEC = 17  # sections per core
NCORES = 8
ROWS_PER_SEC = 256  # padded batch rows per section (2 chunks of 128)
MAIN_W = SECP * OT  # main free width per section (per partition)


def _pattern_groups():
    """(j, istart, gsize) groups of pattern pairs; consecutive moving i."""
    groups = []
    for j in range(BOX):
        ilist = [i for i in range(5) if i != j]
        runs = []
        cur = [ilist[0]]
        for i in ilist[1:]:
            if i == cur[-1] + 1:
                cur.append(i)
            else:
                runs.append(cur)
                cur = [i]
        runs.append(cur)
        for run in runs:
            for cs in range(0, len(run), 3):
                chunk = run[cs : cs + 3]
                groups.append((j, chunk[0], len(chunk)))
    return groups


GROUPS = sorted(_pattern_groups(), key=lambda t: -t[2])  # gsize desc: 3s, 2s, 1s
SLOTS = [(i, j) for (j, i0, g) in GROUPS for i in range(i0, i0 + g)]
assert len(SLOTS) == SECP and len(set(SLOTS)) == SECP

# size classes: (gsize, group_index_start, n_groups, slot_base)
SIZE_CLASSES = []
_gi = 0
_slot = 0
for _gsz in (3, 2, 1):
    _n = sum(1 for (_, _, g) in GROUPS if g == _gsz)
    if _n:
        SIZE_CLASSES.append((_gsz, _gi, _n, _slot))
        _gi += _n
        _slot += _n * _gsz
assert _slot == SECP

# tail layout: per class, groups are banded 4-per-PSUM-bank; the class's tail
# region is ceil(n/4) blocks of width gsize*OT; partition 32*band+o1t.
_tw = 0
CLASS_TAIL_BASE = []
for _gsz, _gi0, _n, _slot0 in SIZE_CLASSES:
    CLASS_TAIL_BASE.append(_tw)
    _tw += ((_n + 3) // 4) * _gsz * OT
TAILW = _tw  # tail free width per section (per partition)


def _tail_maps():
    """Static per-section tail swizzle: for (p, f) in [128, TAILW] ->
    flat element index into a section's [SECP*ROWLEN] row block, or -1."""
    fmap = np.full((128, TAILW), -1, np.int64)
    for ci, (gsz, gi0, n, slot0) in enumerate(SIZE_CLASSES):
        base = CLASS_TAIL_BASE[ci]
        for m in range(n):
            band, block = m % 4, m // 4
            for o1t in range(23):
                p = 32 * band + o1t
                for x in range(gsz):
                    slot = slot0 + m * gsz + x
                    f0 = base + block * gsz * OT + x * OT
                    fmap[p, f0 : f0 + OT] = slot * ROWLEN + (128 + o1t) * OT + np.arange(OT)
    return fmap


TAIL_FMAP = _tail_maps()
TAIL_VALID = TAIL_FMAP >= 0
TAIL_FMAP0 = np.maximum(TAIL_FMAP, 0)


def build_nc(
    nsec=NSEC,
    internal_io=False,
    null_body=False,
    loop_reps=1,
    copy_only=False,
    w_engine="vector",
    no_mm=False,
    no_add=False,
):
    """internal_io=True builds a timing variant: score buffers are Internal
    DRAM (no host transfer), with a tiny external anchor output.
    null_body=True additionally skips the whole section loop.
    loop_reps>1 wraps the body in a hardware For_i loop (timing only).
    copy_only=True strips compute: pure score DMA in/out (calibration)."""
    import concourse.bacc as bacc
    import concourse.tile as tile
    from concourse import mybir
    from contextlib import ExitStack

    f32 = mybir.dt.float32
    bf16 = mybir.dt.float16  # fp16: same PE rate as bf16, 4x less rounding

    nc = bacc.Bacc(None, target_bir_lowering=False)
    io_in = {} if internal_io else {"kind": "ExternalInput"}
    io_out = {} if internal_io else {"kind": "ExternalOutput"}
    score_main = nc.dram_tensor("score_main", [128, nsec * MAIN_W], f32, **io_in)
    score_tail = nc.dram_tensor("score_tail", [128, nsec * TAILW], f32, **io_in)
    meta = nc.dram_tensor(
        "meta", [nsec * ROWS_PER_SEC, 2 * BOX], f32, kind="ExternalInput"
    )
    iota = nc.dram_tensor("iota", [128, OT], f32, kind="ExternalInput")
    out_main = nc.dram_tensor("out_main", [128, nsec * MAIN_W], f32, **io_out)
    out_tail = nc.dram_tensor("out_tail", [128, nsec * TAILW], f32, **io_out)
    anchor = (
        nc.dram_tensor("anchor", [128, OT], f32, kind="ExternalOutput")
        if internal_io
        else None
    )

    with tile.TileContext(nc) as tc, ExitStack() as ctx:
        const_pool = ctx.enter_context(tc.tile_pool(name="const", bufs=1))
        meta_pool = ctx.enter_context(tc.tile_pool(name="meta", bufs=4))
        w_pool = ctx.enter_context(tc.tile_pool(name="w", bufs=6))
        sin_pool = ctx.enter_context(tc.tile_pool(name="sin", bufs=2))
        sout_pool = ctx.enter_context(tc.tile_pool(name="sout", bufs=2))
        tin_pool = ctx.enter_context(tc.tile_pool(name="tin", bufs=4))
        tout_pool = ctx.enter_context(tc.tile_pool(name="tout", bufs=4))
        pm_pool = ctx.enter_context(tc.tile_pool(name="pm", bufs=3, space="PSUM"))
        pt_pool = ctx.enter_context(tc.tile_pool(name="pt", bufs=2, space="PSUM"))

        iota_t = const_pool.tile([128, OT], f32)
        nc.sync.dma_start(iota_t[:], iota[:])
        zeros_t = const_pool.tile([128, 512], f32)
        nc.vector.memset(zeros_t[:], 0.0)

        meta_r = meta.rearrange("(s c r) k -> s r c k", c=2, r=128)

        if anchor is not None:
            nc.sync.dma_start(anchor[:], iota_t[:])

        import contextlib

        loop_ctx = tc.For_i(0, loop_reps, 1) if loop_reps > 1 else contextlib.nullcontext()
        with loop_ctx:
          for s in range(0 if null_body else nsec):
            if copy_only:
                for ci, (g, gi0, ngrp, slot0) in enumerate(SIZE_CLASSES):
                    cw = ngrp * g * OT
                    mbase = s * MAIN_W + slot0 * OT
                    sm = sin_pool.tile([128, cw], f32, tag=f"sin{g}")
                    nc.sync.dma_start(sm[:], score_main[:, mbase : mbase + cw])
                    nc.scalar.dma_start(out_main[:, mbase : mbase + cw], sm[:])
                    tw = ((ngrp + 3) // 4) * g * OT
                    tbase = s * TAILW + CLASS_TAIL_BASE[ci]
                    for b0 in range(0, ngrp, 4):
                        bn = min(4, ngrp - b0)
                        block = b0 // 4
                        bw = g * OT
                        hi = 32 * (bn - 1) + 23
                        fsl = slice(block * bw, (block + 1) * bw)
                        st = tin_pool.tile([128, tw], f32, tag=f"tin{g}")
                        nc.sync.dma_start(
                            st[0:hi, fsl],
                            score_tail[0:hi, tbase + block * bw :][:, 0:bw],
                        )
                        nc.scalar.dma_start(
                            out_tail[0:hi, tbase + block * bw :][:, 0:bw],
                            st[0:hi, fsl],
                        )
                continue
            mt = meta_pool.tile([128, 2, 2 * BOX], f32)
            nc.sync.dma_start(mt[:], meta_r[s])
            w_eng = getattr(nc, w_engine)
            W = []
            for c in range(2):
                w = w_pool.tile([128, BOX, OT], bf16)
                for k in range(BOX):
                    w_eng.tensor_scalar(
                        w[:, k, :],
                        iota_t[:],
                        mt[:, c, k : k + 1],
                        mt[:, c, BOX + k : BOX + k + 1],
                        mybir.AluOpType.is_equal,
                        mybir.AluOpType.mult,
                    )
                W.append(w)

            for ci, (g, gi0, ngrp, slot0) in enumerate(SIZE_CLASSES):
                cw = ngrp * g * OT  # class main width
                mbase = s * MAIN_W + slot0 * OT
                sm = sin_pool.tile([128, cw], f32, tag=f"sin{g}")
                nc.sync.dma_start(sm[:], score_main[:, mbase : mbase + cw])
                om = sout_pool.tile([128, cw], f32, tag=f"sout{g}")

                # ---- mains: clusters of <=2 groups, bank-aligned PSUM ----
                for k0 in range(0, ngrp, 2):
                    kn = min(2, ngrp - k0)
                    psm = None if no_mm else pm_pool.tile([128, kn, 512], f32, tag="pm")
                    if not no_mm:
                      for m in range(kn):
                        j, i0, _ = GROUPS[gi0 + k0 + m]
                        for c in range(2):
                            nc.tensor.matmul(
                                psm[:, m, 0 : g * OT],
                                W[c][:, j, 0:128],
                                W[c][:, i0 : i0 + g, :],
                                start=(c == 0),
                                stop=(c == 1),
                            )
                    if no_add:
                        nc.vector.tensor_copy(
                            om[:, k0 * g * OT : (k0 + kn) * g * OT],
                            sm[:, k0 * g * OT : (k0 + kn) * g * OT],
                        )
                    elif no_mm:
                        nc.vector.tensor_add(
                            om[:, k0 * g * OT : (k0 + kn) * g * OT],
                            sm[:, k0 * g * OT : (k0 + kn) * g * OT],
                            sm[:, k0 * g * OT : (k0 + kn) * g * OT],
                        )
                    else:
                     nc.vector.tensor_add(
                        om[:, k0 * g * OT : (k0 + kn) * g * OT].rearrange(
                            "p (n w) -> p n w", n=kn
                        ),
                        psm[:, :, 0 : g * OT],
                        sm[:, k0 * g * OT : (k0 + kn) * g * OT].rearrange(
                            "p (n w) -> p n w", n=kn
                        ),
                    )
                nc.scalar.dma_start(out_main[:, mbase : mbase + cw], om[:])

                # ---- tails: blocks of 4 groups banded in one PSUM bank ----
                tw = ((ngrp + 3) // 4) * g * OT  # class tail width
                tbase = s * TAILW + CLASS_TAIL_BASE[ci]
                st = tin_pool.tile([128, tw], f32, tag=f"tin{g}")
                ot = tout_pool.tile([128, tw], f32, tag=f"tout{g}")
                for b0 in range(0, ngrp, 4):
                    bn = min(4, ngrp - b0)
                    block = b0 // 4
                    bw = g * OT
                    hi = 32 * (bn - 1) + 23
                    fsl = slice(block * bw, (block + 1) * bw)
                    nc.sync.dma_start(
                        st[0:hi, fsl], score_tail[0:hi, tbase + block * bw :][:, 0:bw]
                    )
                    ptt = None if no_mm else pt_pool.tile([128, 512], f32, tag="pt")
                    if not no_mm:
                      nc.scalar.copy(ptt[0:hi, :], zeros_t[0:hi, :])
                      for m in range(bn):
                        j, i0, _ = GROUPS[gi0 + b0 + m]
                        pb = 32 * m
                        for c in range(2):
                            nc.tensor.matmul(
                                ptt[pb : pb + 23, 0 : g * OT],
                                W[c][:, j, 128:OT],
                                W[c][:, i0 : i0 + g, :],
                                start=False,
                                stop=(c == 1),
                                tile_position=(0, pb),
                                skip_group_check=True,
                            )
                    if no_add:
                        nc.vector.tensor_copy(ot[0:hi, fsl], st[0:hi, fsl])
                    elif no_mm:
                        nc.vector.tensor_add(
                            ot[0:hi, fsl], st[0:hi, fsl], st[0:hi, fsl]
                        )
                    else:
                        nc.vector.tensor_add(
                            ot[0:hi, fsl], ptt[0:hi, 0:bw], st[0:hi, fsl]
                        )
                    nc.scalar.dma_start(
                        out_tail[0:hi, tbase + block * bw :][:, 0:bw], ot[0:hi, fsl]
                    )
    return nc


# ---------------------------------------------------------------------------
# host-side routing
# ---------------------------------------------------------------------------


def _sections():
    secs = [(q, h) for q in range(NUM_QT) for h in (0, 1)]
    secs += [None] * (NCORES * NSEC - len(secs))
    return secs


def _route(obj_label, qus_type, attention, score_matrix):
    score2d = np.ascontiguousarray(score_matrix).reshape(NUM_QT * PAIR, ROWLEN)
    order = np.argsort(qus_type, kind="stable")
    counts = np.bincount(qus_type, minlength=NUM_QT)
    starts = np.concatenate([[0], np.cumsum(counts)])
    secs = _sections()

    iota_arr = np.tile(np.arange(OT, dtype=np.float32), (128, 1))
    in_maps = []
    core_rows = []  # per core: [NSEC*SECP] index into score2d or -1
    for core in range(NCORES):
        sc_rows = np.full(NSEC * SECP, -1, np.int64)
        meta = np.zeros((NSEC * ROWS_PER_SEC, 2 * BOX), np.float32)
        for sl in range(NSEC):
            sec = secs[core * NSEC + sl]
            if sec is None:
                continue
            q, h = sec
            perm = np.array([(x + 5) % 10 if h else x for x in range(BOX)])
            rows = order[starts[q] : starts[q + 1]]
            B = len(rows)
            assert B <= ROWS_PER_SEC, f"group {q} has {B} rows > {ROWS_PER_SEC}"
            meta[sl * ROWS_PER_SEC : sl * ROWS_PER_SEC + B, 0:BOX] = obj_label[rows][
                :, perm
            ].astype(np.float32)
            meta[sl * ROWS_PER_SEC : sl * ROWS_PER_SEC + B, BOX:] = attention[rows][
                :, perm
            ]
            for t, (i, j) in enumerate(SLOTS):
                I, J = perm[i], perm[j]
                p = 9 * I + (J if J < I else J - 1)
                sc_rows[sl * SECP + t] = q * PAIR + p
        full = score2d[np.maximum(sc_rows, 0)]  # [NSEC*SECP, ROWLEN]
        # main: [slot, o1<128, o2] -> [128, slot*OT]
        score_main = np.ascontiguousarray(
            full.reshape(NSEC * SECP, OT, OT)[:, :128, :]
            .transpose(1, 0, 2)
            .reshape(128, NSEC * MAIN_W)
        )
        # tail: banded swizzle per section
        fsec = full.reshape(NSEC, SECP * ROWLEN)
        score_tail = np.zeros((128, NSEC * TAILW), np.float32)
        for sl in range(NSEC):
            vals = fsec[sl][TAIL_FMAP0]
            score_tail[:, sl * TAILW : (sl + 1) * TAILW] = np.where(
                TAIL_VALID, vals, 0.0
            )
        in_maps.append(
            {
                "score_main": score_main,
                "score_tail": score_tail,
                "meta": meta,
                "iota": iota_arr,
            }
        )
        core_rows.append(sc_rows)
    return in_maps, core_rows


def _assemble(results, core_rows):
    """results: list of per-core dicts with out_main/out_tail."""
    out2d = np.empty((NUM_QT * PAIR, ROWLEN), np.float32)
    for core in range(NCORES):
        rows = core_rows[core]
        om = results[core]["out_main"]  # [128, NSEC*MAIN_W]
        ot = results[core]["out_tail"]  # [128, NSEC*TAILW]
        full = np.empty((NSEC * SECP, ROWLEN), np.float32)
        f3 = full.reshape(NSEC * SECP, OT, OT)
        f3[:, :128, :] = om.reshape(128, NSEC * SECP, OT).transpose(1, 0, 2)
        fsec = full.reshape(NSEC, SECP * ROWLEN)
        for sl in range(NSEC):
            blk = ot[:, sl * TAILW : (sl + 1) * TAILW]
            fsec[sl][TAIL_FMAP0[TAIL_VALID]] = blk[TAIL_VALID]
        mask = rows >= 0
        out2d[rows[mask]] = full[mask]
    return out2d.reshape(NUM_QT, PAIR, OT, OT)


_NC_CACHE = {}


def _get_nc(nsec):
    if nsec not in _NC_CACHE:
        nc = build_nc(nsec)
        nc.compile()
        _NC_CACHE[nsec] = nc
    return _NC_CACHE[nsec]


def kernel(obj_label, qus_type, attention, score_matrix):
    from concourse.bass_utils import run_bass_kernel_spmd

    obj_label = np.asarray(obj_label)
    qus_type = np.asarray(qus_type)
    attention = np.asarray(attention, np.float32)
    score_matrix = np.asarray(score_matrix, np.float32)

    in_maps, core_rows = _route(obj_label, qus_type, attention, score_matrix)
    nc = _get_nc(NSEC)
    res = run_bass_kernel_spmd(nc, in_maps, core_ids=list(range(NCORES)))
    return _assemble([res.results[c] for c in range(NCORES)], core_rows)



# revision 4
# speedup vs baseline: 2.0407x; 2.0407x over previous
"""Trainium2 Bass kernel for nn_AccumulatingModule (histogram_binning).

Problem: out = score_matrix.at[qt, p, ol1, ol2].add(at1*at2) — a scatter-add of
BATCH*PAIR outer-product contributions into a [65, 90, 151, 151] fp32 histogram.

Strategy (8 NeuronCores, SPMD) — delta-only device kernel:
  * The additive delta for each (qt, pair) row is a sum of outer products
    W_j^T @ W_i with W[b,k,:] = attention[b,k] * onehot(label[b,k]).  The
    device computes ONLY these dense deltas from the tiny routed meta input
    (~350 KB/core); score_matrix (533 MB) never touches the device.  The
    host adds deltas into a copy of score_matrix at unshard time (the
    "accumulate deltas" step of the expert-routing recipe).
  * Deltas are emitted as bf16: rel error ~2^-9 of the delta only, on top of
    fp16 W rounding -> ~5e-3 worst-case vs the 2e-2 gate.
  * Shard the (qt, half) space: 65 qts x 2 halves = 130 sections + 6 dummies
    = 17 per core.  Pattern pairs {(i,j): j in 5..9, i != j}; the host box
    permutation (identity / +5 mod 10) maps them onto each section's real
    pairs, keeping the compiled kernel identical across cores (SPMD).
  * Per section: 256 padded batch rows = 2 PSUM-accumulated chunks of 128.
    Mains: per j, one 128-col weight load (W_j cols 0..127) streams the 9
    i-blocks (9*151 cols) into a 3-bank PSUM tile.  Tails (o1 in 128..150):
    per chunk ONE packed weight load (tails of all 5 j's = 115 cols) streams
    each W_i once — 151 cols per i instead of a half-rate second pass.
  * PSUM -> SBUF bf16 evacuation split across DVE (mains) and ACT (tails);
    W built on DVE/gpsimd; one [128, 6795] + one [115, 1510] bf16 store per
    section.
"""

import numpy as np

NUM_QT, NUM_OT, PAIR = 65, 151, 90
BOX = 10
OT = NUM_OT
ROWLEN = OT * OT  # 22801
SECP = 45  # pairs per (half) section
NSEC = 17  # sections per core
NCORES = 8
ROWS_PER_SEC = 256  # padded batch rows per section (2 chunks of 128)
PAT_JS = (5, 6, 7, 8, 9)
MAIN_W = SECP * OT  # 6795 = 5 j-blocks * 9 i-slots * 151
TAIL_P = 115  # 5 j's * 23 tail rows
TAIL_W = BOX * OT  # 1510: one 151-col block per i
TGROUPS = ((0, 1, 2), (3, 4, 5), (6, 7, 8), (9,))


def _runs_for(j):
    """Consecutive-i runs of {0..9}\\{j}, each split to <=3 (one PSUM bank)."""
    ilist = [i for i in range(BOX) if i != j]
    runs, cur = [], [ilist[0]]
    for i in ilist[1:]:
        if i == cur[-1] + 1 and len(cur) < 3:
            cur.append(i)
        else:
            runs.append(cur)
            cur = [i]
    runs.append(cur)
    return runs


def _pack_banks(runs):
    """First-fit pack runs into 3 PSUM banks of <=3 i-slots; returns
    [(bank, col_off_elems, run)]."""
    used = [0, 0, 0]
    placement = []
    for r in runs:
        for b in range(3):
            if used[b] + len(r) <= 3:
                placement.append((b, used[b] * OT, r))
                used[b] += len(r)
                break
        else:
            raise RuntimeError("bank packing failed")
    return placement


MAIN_PLACE = {j: _pack_banks(_runs_for(j)) for j in PAT_JS}


def build_nc(
    nsec=NSEC,
    internal_io=False,
    null_body=False,
    loop_reps=1,
    no_mm=False,
    no_dma_out=False,
    dma_only=False,
    w_engine="vector",
):
    """internal_io=True builds a timing variant: out buffers are Internal
    DRAM (no host transfer), with a tiny external anchor output.
    null_body=True additionally skips the whole section loop.
    loop_reps>1 wraps the body in a hardware For_i loop (timing only).
    Attribution variants: no_mm (skip PE, evac copies zeros), no_dma_out
    (compute only), dma_only (just the out stores from a constant tile)."""
    import concourse.bacc as bacc
    import concourse.tile as tile
    from concourse import mybir
    from contextlib import ExitStack
    import contextlib

    f32 = mybir.dt.float32
    f16 = mybir.dt.float16  # W dtype: one-hot exact, attention rounded once
    bf16 = mybir.dt.bfloat16  # delta transport dtype

    nc = bacc.Bacc(None, target_bir_lowering=False)
    io_out = {} if internal_io else {"kind": "ExternalOutput"}
    meta = nc.dram_tensor(
        "meta", [nsec * ROWS_PER_SEC, 2 * BOX], f32, kind="ExternalInput"
    )
    iota = nc.dram_tensor("iota", [128, OT], f32, kind="ExternalInput")
    out_main = nc.dram_tensor("out_main", [128, nsec * MAIN_W], bf16, **io_out)
    out_tail = nc.dram_tensor("out_tail", [TAIL_P, nsec * TAIL_W], bf16, **io_out)
    anchor = (
        nc.dram_tensor("anchor", [128, OT], f32, kind="ExternalOutput")
        if internal_io
        else None
    )

    with tile.TileContext(nc) as tc, ExitStack() as ctx:
        const_pool = ctx.enter_context(tc.tile_pool(name="const", bufs=1))
        meta_pool = ctx.enter_context(tc.tile_pool(name="meta", bufs=4))
        w_pool = ctx.enter_context(tc.tile_pool(name="w", bufs=3))
        om_pool = ctx.enter_context(tc.tile_pool(name="om", bufs=3))
        ot_pool = ctx.enter_context(tc.tile_pool(name="ot", bufs=3))
        pm_pool = ctx.enter_context(tc.tile_pool(name="pm", bufs=2, space="PSUM"))
        pt_pool = ctx.enter_context(tc.tile_pool(name="pt", bufs=2, space="PSUM"))

        iota_t = const_pool.tile([128, OT], f32)
        nc.sync.dma_start(iota_t[:], iota[:])
        if anchor is not None:
            nc.sync.dma_start(anchor[:], iota_t[:])
        if no_mm or dma_only:
            zmain = const_pool.tile([128, MAIN_W], bf16)
            nc.vector.memset(zmain[:], 0.0)
            ztail = const_pool.tile([128, TAIL_W], bf16)
            nc.vector.memset(ztail[:], 0.0)

        meta_r = meta.rearrange("(s c r) k -> s r c k", c=2, r=128)
        w_eng = getattr(nc, w_engine)

        loop_ctx = tc.For_i(0, loop_reps, 1) if loop_reps > 1 else contextlib.nullcontext()
        with loop_ctx:
          for s in range(0 if null_body else nsec):
            if dma_only:
                nc.scalar.dma_start(
                    out_main[:, s * MAIN_W : (s + 1) * MAIN_W], zmain[:]
                )
                nc.sync.dma_start(
                    out_tail[:, s * TAIL_W : (s + 1) * TAIL_W],
                    ztail[0:TAIL_P, :],
                )
                continue

            mt = meta_pool.tile([128, 2, 2 * BOX], f32)
            nc.sync.dma_start(mt[:], meta_r[s])

            # ---- W build: [128, 2, BOX, OT] + packed tails [128, 2, 115] ----
            w = w_pool.tile([128, 2, BOX, OT], f16, tag="w")
            wt = w_pool.tile([128, 2, TAIL_P], f16, tag="wt")
            if not no_mm:
                for c in range(2):
                    for k in range(BOX):
                        w_eng.tensor_scalar(
                            w[:, c, k, :],
                            iota_t[:],
                            mt[:, c, k : k + 1],
                            mt[:, c, BOX + k : BOX + k + 1],
                            mybir.AluOpType.is_equal,
                            mybir.AluOpType.mult,
                        )
                    for jt, j in enumerate(PAT_JS):
                        w_eng.tensor_scalar(
                            wt[:, c, jt * 23 : (jt + 1) * 23],
                            iota_t[:, 128:OT],
                            mt[:, c, j : j + 1],
                            mt[:, c, BOX + j : BOX + j + 1],
                            mybir.AluOpType.is_equal,
                            mybir.AluOpType.mult,
                        )

            om = om_pool.tile([128, MAIN_W], bf16, tag="om")
            otl = ot_pool.tile([128, TAIL_W], bf16, tag="ot")

            # ---- interleave tail groups and main j-blocks ----
            blocks = []
            for tg in TGROUPS:
                blocks.append(("tail", tg))
            for j in PAT_JS:
                blocks.append(("main", j))
            order = []
            for x in range(5):
                if x < len(TGROUPS):
                    order.append(blocks[x])
                order.append(blocks[4 + x])

            for kind, arg in order:
                if kind == "tail":
                    tg = arg
                    tbase = tg[0] * OT
                    tw = len(tg) * OT
                    if no_mm:
                        nc.scalar.copy(
                            otl[0:TAIL_P, tbase : tbase + tw],
                            ztail[0:TAIL_P, tbase : tbase + tw],
                        )
                        continue
                    ptt = pt_pool.tile([128, 512], f32, tag="pt")
                    for c in range(2):
                        for si, i in enumerate(tg):
                            # start=True clears has_written for the WHOLE bank,
                            # so only the first matmul into the bank may set it;
                            # later regions overwrite-on-unset automatically.
                            nc.tensor.matmul(
                                ptt[0:TAIL_P, si * OT : (si + 1) * OT],
                                wt[:, c, :],
                                w[:, c, i, :],
                                start=(c == 0 and si == 0),
                                stop=(c == 1),
                                skip_group_check=True,
                            )
                    nc.scalar.copy(
                        otl[0:TAIL_P, tbase : tbase + tw], ptt[0:TAIL_P, 0:tw]
                    )
                else:
                    j = arg
                    jb = (j - 5) * 9 * OT
                    if no_mm:
                        nc.vector.tensor_copy(
                            om[:, jb : jb + 9 * OT], zmain[:, jb : jb + 9 * OT]
                        )
                        continue
                    psm = pm_pool.tile([128, 3, 512], f32, tag="pm")
                    for c in range(2):
                        seen_banks = set()
                        for b, coff, run in MAIN_PLACE[j]:
                            first_in_bank = b not in seen_banks
                            seen_banks.add(b)
                            # bank-wide has_written clear: start only on the
                            # first matmul per bank (see tails comment).
                            nc.tensor.matmul(
                                psm[:, b, coff : coff + len(run) * OT],
                                w[:, c, j, 0:128],
                                w[:, c, run[0] : run[0] + len(run), :],
                                start=(c == 0 and first_in_bank),
                                stop=(c == 1),
                                skip_group_check=True,
                            )
                    for b, coff, run in MAIN_PLACE[j]:
                        slot = run[0] - (run[0] > j)
                        nc.vector.tensor_copy(
                            om[:, jb + slot * OT : jb + (slot + len(run)) * OT],
                            psm[:, b, coff : coff + len(run) * OT],
                        )

            if not no_dma_out:
                nc.scalar.dma_start(
                    out_main[:, s * MAIN_W : (s + 1) * MAIN_W], om[:]
                )
                nc.sync.dma_start(
                    out_tail[:, s * TAIL_W : (s + 1) * TAIL_W], otl[0:TAIL_P, :]
                )
    return nc


# ---------------------------------------------------------------------------
# host-side routing
# ---------------------------------------------------------------------------


def _sections():
    secs = [(q, h) for q in range(NUM_QT) for h in (0, 1)]
    secs += [None] * (NCORES * NSEC - len(secs))
    return secs


def _route(obj_label, qus_type, attention):
    order = np.argsort(qus_type, kind="stable")
    counts = np.bincount(qus_type, minlength=NUM_QT)
    starts = np.concatenate([[0], np.cumsum(counts)])
    secs = _sections()

    iota_arr = np.tile(np.arange(OT, dtype=np.float32), (128, 1))
    in_maps = []
    for core in range(NCORES):
        meta = np.zeros((NSEC * ROWS_PER_SEC, 2 * BOX), np.float32)
        for sl in range(NSEC):
            sec = secs[core * NSEC + sl]
            if sec is None:
                continue
            q, h = sec
            perm = np.array([(x + 5) % 10 if h else x for x in range(BOX)])
            rows = order[starts[q] : starts[q + 1]]
            B = len(rows)
            assert B <= ROWS_PER_SEC, f"group {q} has {B} rows > {ROWS_PER_SEC}"
            meta[sl * ROWS_PER_SEC : sl * ROWS_PER_SEC + B, 0:BOX] = obj_label[rows][
                :, perm
            ].astype(np.float32)
            meta[sl * ROWS_PER_SEC : sl * ROWS_PER_SEC + B, BOX:] = attention[rows][
                :, perm
            ]
        in_maps.append({"meta": meta, "iota": iota_arr})
    return in_maps


def _assemble(results, score_matrix):
    """results: per-core dicts with out_main [128, NSEC*MAIN_W] bf16 and
    out_tail [115, NSEC*TAIL_W] bf16.  Returns score + delta, fp32."""
    out2d = np.ascontiguousarray(score_matrix, np.float32).reshape(
        NUM_QT * PAIR, ROWLEN
    ).copy()
    secs = _sections()
    for core in range(NCORES):
        om = np.asarray(results[core]["out_main"], np.float32)
        otl = np.asarray(results[core]["out_tail"], np.float32)
        for sl in range(NSEC):
            sec = secs[core * NSEC + sl]
            if sec is None:
                continue
            q, h = sec
            perm = np.array([(x + 5) % 10 if h else x for x in range(BOX)])
            dm = om[:, sl * MAIN_W : (sl + 1) * MAIN_W].reshape(128, 5, 9, OT)
            dt = otl[:, sl * TAIL_W : (sl + 1) * TAIL_W].reshape(5, 23, BOX, OT)
            rows = np.empty(SECP, np.int64)
            delta = np.empty((SECP, OT, OT), np.float32)
            t = 0
            for jb in range(5):
                j = 5 + jb
                for sI in range(9):
                    i = sI if sI < j else sI + 1
                    I, J = perm[i], perm[j]
                    p = 9 * I + (J if J < I else J - 1)
                    rows[t] = q * PAIR + p
                    delta[t, 0:128, :] = dm[:, jb, sI, :]
                    delta[t, 128:OT, :] = dt[jb, :, i, :]
                    t += 1
            out2d[rows] += delta.reshape(SECP, ROWLEN)
    return out2d.reshape(NUM_QT, PAIR, OT, OT)


_NC_CACHE = {}


def _get_nc(nsec):
    if nsec not in _NC_CACHE:
        nc = build_nc(nsec)
        nc.compile()
        _NC_CACHE[nsec] = nc
    return _NC_CACHE[nsec]


def kernel(obj_label, qus_type, attention, score_matrix):
    from concourse.bass_utils import run_bass_kernel_spmd

    obj_label = np.asarray(obj_label)
    qus_type = np.asarray(qus_type)
    attention = np.asarray(attention, np.float32)
    score_matrix = np.asarray(score_matrix, np.float32)

    in_maps = _route(obj_label, qus_type, attention)
    nc = _get_nc(NSEC)
    res = run_bass_kernel_spmd(nc, in_maps, core_ids=list(range(NCORES)))
    return _assemble([res.results[c] for c in range(NCORES)], score_matrix)


# revision 14
# speedup vs baseline: 2.4409x; 1.1961x over previous
"""Trainium2 Bass kernel for nn_AccumulatingModule (histogram_binning).

Problem: out = score_matrix.at[qt, p, ol1, ol2].add(at1*at2) — a scatter-add of
BATCH*PAIR outer-product contributions into a [65, 90, 151, 151] fp32 histogram.

Strategy (8 NeuronCores, SPMD) — delta-only device kernel:
  * The additive delta for each (qt, pair) row is a sum of outer products
    W_j^T @ W_i with W[b,k,:] = attention[b,k] * onehot(label[b,k]).  The
    device computes ONLY these dense deltas from the tiny routed meta input
    (~350 KB/core); score_matrix (533 MB) never touches the device.  The
    host adds deltas into a copy of score_matrix at unshard time (the
    "accumulate deltas" step of the expert-routing recipe).
  * Deltas are emitted as bf16: rel error ~2^-9 of the delta only, on top of
    fp16 W rounding -> ~5e-3 worst-case vs the 2e-2 gate.
  * Shard the (qt, half) space: 65 qts x 2 halves = 130 sections + 6 dummies
    = 17 per core.  Pattern pairs {(i,j): j in 5..9, i != j}; the host box
    permutation (identity / +5 mod 10) maps them onto each section's real
    pairs, keeping the compiled kernel identical across cores (SPMD).
  * Mixed chunking: the first N2=9 slots/core PSUM-accumulate 2 chunks of
    128 batch rows; the rest are single-chunk.  The router sends qts with
    >128 rows to 2-chunk slots (54 of 72 used at seed distribution).
  * Mains: per j, one 128-col weight load (W_j cols 0..127) streams the 9
    i-blocks in i-slot order into bank-aligned PSUM (2-bank + 1-bank tiles)
    so evacuation is one big strided copy per tile.  Tails (o1 128..150):
    ONE packed strided weight load (tails of all 5 j's = 115 cols) streams
    each W_i once — 151 cols per i instead of a half-rate second pass.
  * W built on DVE from an fp16 iota (all-2-byte operands -> DVE 4x mode);
    PSUM evacuation split DVE/ACT; per-section stores go to section-major
    contiguous DRAM blocks, alternating the two HWDGE rings.
"""

import numpy as np

NUM_QT, NUM_OT, PAIR = 65, 151, 90
BOX = 10
OT = NUM_OT
ROWLEN = OT * OT  # 22801
SECP = 45  # pairs per (half) section
NSEC = 17  # sections per core
N2 = 9  # 2-chunk slots per core (rest are 1-chunk)
NCORES = 8
ROWS_PER_SEC = 256  # meta rows per section slot (2-chunk slots use all 256)
PAT_JS = (5, 6, 7, 8, 9)
MAIN_W = SECP * OT  # 6795 = 5 j-blocks * 9 i-slots * 151
TAIL_P = 115  # 5 j's * 23 tail rows
TAIL_W = BOX * OT  # 1510: one 151-col block per i


def _slot_runs(j):
    """i-slot-ordered matmul runs for j's 9 i-values.  Slot s holds
    i = s + (s >= j); slots are grouped 3 per PSUM bank (bank = s//3,
    col = (s%3)*OT) so evacuation is one contiguous copy per 453-col bank.
    Returns [(bank, col_off, i0, glen)]."""
    out = []
    for t in range(3):
        run = []  # list of (slot, i)
        for s in range(3 * t, 3 * t + 3):
            i = s + (1 if s >= j else 0)
            if run and i != run[-1][1] + 1:
                out.append((t, (run[0][0] % 3) * OT, run[0][1], len(run)))
                run = []
            run.append((s, i))
        out.append((t, (run[0][0] % 3) * OT, run[0][1], len(run)))
    return out


MAIN_PLACE = {j: _slot_runs(j) for j in PAT_JS}
# tails: two phases; each phase = one [128, 2, 512] PSUM tile, i-slots 3/bank
TAIL_PHASES = (((0, 1, 2), (3, 4, 5)), ((6, 7, 8), (9,)))


def _chunks_of(sl):
    return 2 if sl < N2 else 1


def build_nc(
    nsec=NSEC,
    internal_io=False,
    null_body=False,
    loop_reps=1,
    no_mm=False,
    no_dma_out=False,
    dma_only=False,
    no_evac=False,
    w_only=False,
    tail_pack=True,
):
    """internal_io=True builds a timing variant: out buffers are Internal
    DRAM (no host transfer), with a tiny external anchor output.
    null_body=True additionally skips the whole section loop.
    loop_reps>1 wraps the body in a hardware For_i loop (timing only).
    Attribution variants: no_mm (skip PE+W, copies from zeros), no_dma_out,
    dma_only, no_evac (PE+W only), w_only (W build only).
    tail_pack=True builds an explicitly packed tail-weight tile instead of
    the strided view into w."""
    import concourse.bacc as bacc
    import concourse.tile as tile
    from concourse import mybir
    from contextlib import ExitStack
    import contextlib

    f32 = mybir.dt.float32
    f16 = mybir.dt.float16  # W dtype: one-hot exact, attention rounded once
    bf16 = mybir.dt.bfloat16  # delta transport dtype

    nc = bacc.Bacc(None, target_bir_lowering=False)
    io_out = {} if internal_io else {"kind": "ExternalOutput"}
    meta = nc.dram_tensor(
        "meta", [nsec * ROWS_PER_SEC, 2 * BOX], f32, kind="ExternalInput"
    )
    iota = nc.dram_tensor("iota", [128, OT], f16, kind="ExternalInput")
    out_main = nc.dram_tensor("out_main", [nsec * 128, MAIN_W], bf16, **io_out)
    out_tail = nc.dram_tensor("out_tail", [nsec * TAIL_P, TAIL_W], bf16, **io_out)
    anchor = (
        nc.dram_tensor("anchor", [128, OT], f16, kind="ExternalOutput")
        if internal_io
        else None
    )

    with tile.TileContext(nc) as tc, ExitStack() as ctx:
        const_pool = ctx.enter_context(tc.tile_pool(name="const", bufs=1))
        meta_pool = ctx.enter_context(tc.tile_pool(name="meta", bufs=4))
        w_pool = ctx.enter_context(tc.tile_pool(name="w", bufs=3))
        om_pool = ctx.enter_context(tc.tile_pool(name="om", bufs=3))
        ot_pool = ctx.enter_context(tc.tile_pool(name="ot", bufs=3))
        pa_pool = ctx.enter_context(tc.tile_pool(name="pa", bufs=2, space="PSUM"))
        pb_pool = ctx.enter_context(tc.tile_pool(name="pb", bufs=2, space="PSUM"))
        pt_pool = ctx.enter_context(tc.tile_pool(name="pt", bufs=1, space="PSUM"))

        iota_t = const_pool.tile([128, OT], f16)
        nc.sync.dma_start(iota_t[:], iota[:])
        if anchor is not None:
            nc.sync.dma_start(anchor[:], iota_t[:])
        if no_mm or dma_only:
            zmain = const_pool.tile([128, MAIN_W], bf16)
            nc.vector.memset(zmain[:], 0.0)
            ztail = const_pool.tile([128, 2 * 906], bf16)
            nc.vector.memset(ztail[:], 0.0)

        meta_r = meta.rearrange("(s c r) k -> s r c k", c=2, r=128)

        loop_ctx = tc.For_i(0, loop_reps, 1) if loop_reps > 1 else contextlib.nullcontext()
        with loop_ctx:
          for s in range(0 if null_body else nsec):
            nch = _chunks_of(s)
            om_dma, ot_dma = (
                (nc.scalar, nc.sync) if s % 2 == 0 else (nc.sync, nc.scalar)
            )
            if dma_only:
                om_dma.dma_start(out_main[s * 128 : (s + 1) * 128, :], zmain[:])
                ot_dma.dma_start(
                    out_tail[s * TAIL_P : (s + 1) * TAIL_P, :],
                    ztail[0:TAIL_P, 0:TAIL_W],
                )
                continue

            mt = meta_pool.tile([128, 2, 2 * BOX], f32)
            nc.sync.dma_start(mt[:, 0:nch, :], meta_r[s][:, 0:nch, :])

            # ---- W build on DVE: [128, nch, BOX, OT] fp16 (4x mode) ----
            w = w_pool.tile([128, 2, BOX, OT], f16, tag="w")
            if tail_pack:
                wt = w_pool.tile([128, 2, TAIL_P], f16, tag="wt")
            if not (no_mm or dma_only):
                for c in range(nch):
                    for k in range(BOX):
                        nc.vector.tensor_scalar(
                            w[:, c, k, :],
                            iota_t[:],
                            mt[:, c, k : k + 1],
                            mt[:, c, BOX + k : BOX + k + 1],
                            mybir.AluOpType.is_equal,
                            mybir.AluOpType.mult,
                        )
                    if tail_pack:
                        for jt, j in enumerate(PAT_JS):
                            nc.vector.tensor_scalar(
                                wt[:, c, jt * 23 : (jt + 1) * 23],
                                iota_t[:, 128:OT],
                                mt[:, c, j : j + 1],
                                mt[:, c, BOX + j : BOX + j + 1],
                                mybir.AluOpType.is_equal,
                                mybir.AluOpType.mult,
                            )
            if w_only:
                continue

            om = om_pool.tile([128, MAIN_W], bf16, tag="om")
            otl = ot_pool.tile([128, 2 * 906], bf16, tag="ot")

            def tail_lhs(c):
                return wt[:, c, :] if tail_pack else w[:, c, 5:BOX, 128:OT]

            # ---- interleaved: tail phase, then mains (tails first so their
            # single-buffered psum tile frees early) ----
            for ph, (kind, arg) in enumerate(
                [("tail", 0), ("main", 5), ("main", 6), ("tail", 1),
                 ("main", 7), ("main", 8), ("main", 9)]
            ):
                if kind == "tail":
                    phase = TAIL_PHASES[arg]
                    obase = arg * 906
                    if no_mm:
                        nc.scalar.copy(
                            otl[0:TAIL_P, obase : obase + 906],
                            ztail[0:TAIL_P, obase : obase + 906],
                        )
                        continue
                    ptt = pt_pool.tile([128, 2, 512], f32, tag="pt")
                    for c in range(nch):
                        for b, slots in enumerate(phase):
                            for si, i in enumerate(slots):
                                # start=True clears has_written for the WHOLE
                                # bank: set it only on the bank's first matmul;
                                # later regions overwrite-on-unset.
                                nc.tensor.matmul(
                                    ptt[0:TAIL_P, b, si * OT : (si + 1) * OT],
                                    tail_lhs(c),
                                    w[:, c, i, :],
                                    start=(c == 0 and si == 0),
                                    stop=(c == nch - 1),
                                    skip_group_check=True,
                                )
                    eng = nc.vector if arg == 0 else nc.scalar
                    if eng is nc.vector:
                        eng.tensor_copy(
                            otl[0:TAIL_P, obase : obase + 906],
                            ptt[0:TAIL_P, :, 0:453],
                        )
                    else:
                        eng.copy(
                            otl[0:TAIL_P, obase : obase + 906],
                            ptt[0:TAIL_P, :, 0:453],
                        )
                else:
                    j = arg
                    jb = (j - 5) * 9 * OT
                    if no_mm:
                        nc.vector.tensor_copy(
                            om[:, jb : jb + 9 * OT], zmain[:, jb : jb + 9 * OT]
                        )
                        continue
                    psa = pa_pool.tile([128, 2, 512], f32, tag="pa")
                    psb = pb_pool.tile([128, 512], f32, tag="pb")
                    for c in range(nch):
                        seen = set()
                        for b, coff, i0, glen in MAIN_PLACE[j]:
                            dst = (
                                psa[:, b, coff : coff + glen * OT]
                                if b < 2
                                else psb[:, coff : coff + glen * OT]
                            )
                            nc.tensor.matmul(
                                dst,
                                w[:, c, j, 0:128],
                                w[:, c, i0 : i0 + glen, :],
                                start=(c == 0 and b not in seen),
                                stop=(c == nch - 1),
                                skip_group_check=True,
                            )
                            seen.add(b)
                    if no_evac:
                        continue
                    # slots 0..5 from the 2-bank tile on ACT, 6..8 on DVE
                    nc.scalar.copy(om[:, jb : jb + 906], psa[:, :, 0:453])
                    nc.vector.tensor_copy(
                        om[:, jb + 906 : jb + 1359], psb[:, 0:453]
                    )

            if not (no_dma_out or no_evac):
                om_dma.dma_start(out_main[s * 128 : (s + 1) * 128, :], om[:])
                ot_dma.dma_start(
                    out_tail[s * TAIL_P : (s + 1) * TAIL_P, :],
                    otl[0:TAIL_P, 0:TAIL_W],
                )
    return nc


# ---------------------------------------------------------------------------
# host-side routing
# ---------------------------------------------------------------------------


def _route(obj_label, qus_type, attention):
    """Returns (in_maps, placement) where placement[core][slot] =
    (q, h) or None."""
    order = np.argsort(qus_type, kind="stable")
    counts = np.bincount(qus_type, minlength=NUM_QT)
    starts = np.concatenate([[0], np.cumsum(counts)])

    big_qs = [q for q in range(NUM_QT) if counts[q] > 128]
    small_qs = [q for q in range(NUM_QT) if counts[q] <= 128]
    assert counts.max() <= ROWS_PER_SEC, f"qt group of {counts.max()} rows"
    big = [(q, h) for q in big_qs for h in (0, 1)]
    small = [(q, h) for q in small_qs for h in (0, 1)]
    assert len(big) <= NCORES * N2, (
        f"{len(big)} two-chunk sections exceed capacity {NCORES * N2}"
    )
    # fill 2-chunk slots with big sections (round-robin over cores), then
    # spill small sections into leftover 2-chunk slots, then 1-chunk slots.
    placement = [[None] * NSEC for _ in range(NCORES)]
    slots2 = [(c, sl) for sl in range(N2) for c in range(NCORES)]
    slots1 = [(c, sl) for sl in range(N2, NSEC) for c in range(NCORES)]
    pool = big + small
    for (c, sl), sec in zip(slots2 + slots1, pool + [None] * 99):
        placement[c][sl] = sec

    iota_arr = np.tile(np.arange(OT, dtype=np.float16), (128, 1))
    in_maps = []
    for core in range(NCORES):
        meta = np.zeros((NSEC * ROWS_PER_SEC, 2 * BOX), np.float32)
        for sl in range(NSEC):
            sec = placement[core][sl]
            if sec is None:
                continue
            q, h = sec
            perm = np.array([(x + 5) % 10 if h else x for x in range(BOX)])
            rows = order[starts[q] : starts[q + 1]]
            B = len(rows)
            assert B <= 128 * _chunks_of(sl)
            meta[sl * ROWS_PER_SEC : sl * ROWS_PER_SEC + B, 0:BOX] = obj_label[rows][
                :, perm
            ].astype(np.float32)
            meta[sl * ROWS_PER_SEC : sl * ROWS_PER_SEC + B, BOX:] = attention[rows][
                :, perm
            ]
        in_maps.append({"meta": meta, "iota": iota_arr})
    return in_maps, placement


def _assemble(results, placement, score_matrix):
    """results: per-core dicts with out_main [NSEC*128, MAIN_W] bf16 and
    out_tail [NSEC*115, TAIL_W] bf16.  Returns score + delta, fp32."""
    out2d = np.ascontiguousarray(score_matrix, np.float32).reshape(
        NUM_QT * PAIR, ROWLEN
    ).copy()
    for core in range(NCORES):
        om = np.asarray(results[core]["out_main"], np.float32)
        otl = np.asarray(results[core]["out_tail"], np.float32)
        for sl in range(NSEC):
            sec = placement[core][sl]
            if sec is None:
                continue
            q, h = sec
            perm = np.array([(x + 5) % 10 if h else x for x in range(BOX)])
            dm = om[sl * 128 : (sl + 1) * 128].reshape(128, 5, 9, OT)
            dt = otl[sl * TAIL_P : (sl + 1) * TAIL_P].reshape(5, 23, BOX, OT)
            rows = np.empty(SECP, np.int64)
            delta = np.empty((SECP, OT, OT), np.float32)
            t = 0
            for jb in range(5):
                j = 5 + jb
                for sI in range(9):
                    i = sI if sI < j else sI + 1
                    I, J = perm[i], perm[j]
                    p = 9 * I + (J if J < I else J - 1)
                    rows[t] = q * PAIR + p
                    delta[t, 0:128, :] = dm[:, jb, sI, :]
                    delta[t, 128:OT, :] = dt[jb, :, i, :]
                    t += 1
            out2d[rows] += delta.reshape(SECP, ROWLEN)
    return out2d.reshape(NUM_QT, PAIR, OT, OT)


_NC_CACHE = {}


def _get_nc(nsec):
    if nsec not in _NC_CACHE:
        nc = build_nc(nsec)
        nc.compile()
        _NC_CACHE[nsec] = nc
    return _NC_CACHE[nsec]


def kernel(obj_label, qus_type, attention, score_matrix):
    from concourse.bass_utils import run_bass_kernel_spmd

    obj_label = np.asarray(obj_label)
    qus_type = np.asarray(qus_type)
    attention = np.asarray(attention, np.float32)
    score_matrix = np.asarray(score_matrix, np.float32)

    in_maps, placement = _route(obj_label, qus_type, attention)
    nc = _get_nc(NSEC)
    res = run_bass_kernel_spmd(nc, in_maps, core_ids=list(range(NCORES)))
    return _assemble(
        [res.results[c] for c in range(NCORES)], placement, score_matrix
    )
